# Initial kernel scaffold
#
"""Trainium2 Bass kernel for nn_ClippedGRU: B=128, T=1024, I=256, H=512, clip=5.0.

Strategy (8-way data-parallel, B=16 per core, no collectives):
  All on-chip state lives in "transposed" layout: gates/hidden on SBUF
  partitions, (batch, time) on the free dim.

  Phase 1: gi.T = w_ih @ x.T as one big PE GEMM (fp16 in, fp32 PSUM),
  staged to an HBM scratch in per-t-block tiles [128, 12, (16b x 32t)].
  x.T comes from xbar DMA-transpose (fp16).

  Phase 2: 1024 serial GRU steps. Per step:
    - 3 tiny "bias inject" matmuls put b_hh/b_ih into PSUM (selector trick)
    - 1-2 "identity inject" matmuls accumulate gi_t into PSUM
    - 48 LDWEIGHTS+MATMUL pairs (w_hh.T tiles fp16, N=16) -> gates PSUM
    - ACT sigmoid/tanh + DVE elementwise chain -> h (fp32 master + fp16 copy)
    - PE transpose of h -> output rows staged [64, 32t, 128] and DMA'd per block
"""

import sys
import os

for _p in ("/opt/trn_rl_repo", "/root/.axon_site/_ro/trn_rl_repo"):
    if os.path.isdir(_p) and _p not in sys.path:
        sys.path.insert(0, _p)

import numpy as np

import concourse.bass as bass
import concourse.tile as tile
from concourse import bacc, mybir
from concourse.bass import ds, ts
from concourse.bass_utils import run_bass_kernel_spmd

F16 = mybir.dt.float16
F32 = mybir.dt.float32
AF = mybir.ActivationFunctionType
OP = mybir.AluOpType

B_FULL, T, I, H = 128, 1024, 256, 512
N_CORES = 8
B = B_FULL // N_CORES        # 16 per core
TBS = 32                     # timesteps per t-block
NTB = T // TBS               # 32 t-blocks
NG = 12                      # gate chunks of 128 (3H/128)
NK = H // 128                # 4 k-chunks of hidden
CLIP = 5.0

_cache = {}


def _emit_phase1(nc, tc, ctx, x16, wih, gi_hbm):
    """gi.T = w_ih @ x.T  ->  gi_hbm[tb, p, j, (b t)]  (fp16)."""
    import contextlib

    with contextlib.ExitStack() as p1:
        pool = p1.enter_context(tc.tile_pool(name="p1sbuf", bufs=1))
        tmp = p1.enter_context(tc.tile_pool(name="p1tmp", bufs=4))
        psum = p1.enter_context(tc.tile_pool(name="p1psum", bufs=4, space="PSUM"))

        wih_sb = pool.tile([128, 2, 3 * H], F16)
        nc.sync.dma_start(wih_sb[:], wih[:])

        xT = pool.tile([128, 2, B * T], F16)
        for b in range(B):
            for k in range(2):
                nc.sync.dma_start_transpose(
                    xT[:, k, b * T : (b + 1) * T],
                    x16[b, :, k * 128 : (k + 1) * 128],
                )

        xTr = [xT[:, k].rearrange("p (b t) -> p b t", b=B) for k in range(2)]
        for tb in range(NTB):
            for j in range(NG):
                ps = psum.tile([128, B * TBS], F32, tag="p1ps")
                for k in range(2):
                    nc.tensor.matmul(
                        ps[:],
                        wih_sb[:, k, ts(j, 128)],
                        xTr[k][:, :, ts(tb, TBS)],
                        start=(k == 0),
                        stop=(k == 1),
                    )
                gi_t = tmp.tile([128, B * TBS], F16, tag="p1gi")
                if j % 2 == 0:
                    nc.scalar.activation(gi_t[:], ps[:], AF.Copy)
                else:
                    nc.vector.tensor_copy(gi_t[:], ps[:])
                nc.sync.dma_start(gi_hbm[tb, :, j, :], gi_t[:])


def build(nc):
    x16 = nc.declare_dram_parameter("x16", [B, T, I], F16, isOutput=False)
    h0p = nc.declare_dram_parameter("h0p", [128, NK], F32, isOutput=False)
    whh = nc.declare_dram_parameter("whh", [128, 48, 128], F16, isOutput=False)
    wih = nc.declare_dram_parameter("wih", [128, 2, 3 * H], F16, isOutput=False)
    b_rz = nc.declare_dram_parameter("b_rz", [8, 128], F16, isOutput=False)
    b_nh = nc.declare_dram_parameter("b_nh", [4, 128], F16, isOutput=False)
    b_ni = nc.declare_dram_parameter("b_ni", [4, 128], F16, isOutput=False)
    sel8 = nc.declare_dram_parameter("sel8", [8, 128], F16, isOutput=False)
    sel4 = nc.declare_dram_parameter("sel4", [4, 64], F16, isOutput=False)
    ident = nc.declare_dram_parameter("ident", [128, 128], F16, isOutput=False)
    out = nc.declare_dram_parameter("out", [B, T, H], F32, isOutput=True)

    gi_hbm = nc.dram_tensor("gi_scratch", [NTB, 128, NG, B * TBS], F16)

    # output viewed as [c, b, t, p] where h = c*128 + p
    out_cbtp = out.ap().rearrange("b t (c p) -> c b t p", p=128)

    with tile.TileContext(nc) as tc:
        import contextlib

        ctx = contextlib.ExitStack()
        with ctx:
            _emit_phase1(nc, tc, ctx, x16, wih, gi_hbm)

            singles = ctx.enter_context(tc.tile_pool(name="singles", bufs=1))
            chain = ctx.enter_context(tc.tile_pool(name="chain", bufs=2))
            obuf = ctx.enter_context(tc.tile_pool(name="obuf", bufs=2))
            ps_rz = ctx.enter_context(tc.tile_pool(name="ps_rz", bufs=2, space="PSUM"))
            ps_n = ctx.enter_context(tc.tile_pool(name="ps_n", bufs=2, space="PSUM"))
            ps_n2 = ctx.enter_context(tc.tile_pool(name="ps_n2", bufs=2, space="PSUM"))
            ps_tr = ctx.enter_context(tc.tile_pool(name="ps_tr", bufs=2, space="PSUM"))

            whh_sb = singles.tile([128, 48, 128], F16)
            nc.sync.dma_start(whh_sb[:], whh[:])
            ident_sb = singles.tile([128, 128], F16)
            nc.sync.dma_start(ident_sb[:], ident[:])
            sel8_sb = singles.tile([8, 128], F16)
            nc.sync.dma_start(sel8_sb[:], sel8[:])
            sel4_sb = singles.tile([4, 64], F16)
            nc.sync.dma_start(sel4_sb[:], sel4[:])
            b_rz_sb = singles.tile([8, 128], F16)
            nc.sync.dma_start(b_rz_sb[:], b_rz[:])
            b_nh_sb = singles.tile([4, 128], F16)
            nc.sync.dma_start(b_nh_sb[:], b_nh[:])
            b_ni_sb = singles.tile([4, 128], F16)
            nc.sync.dma_start(b_ni_sb[:], b_ni[:])

            h16 = singles.tile([128, NK, B], F16)
            h32 = singles.tile([128, NK, B], F32)
            h0p_sb = singles.tile([128, NK], F32)
            nc.sync.dma_start(h0p_sb[:], h0p[:])
            nc.vector.tensor_copy(h32[:], h0p_sb[:, :, None].to_broadcast([128, NK, B]))
            nc.vector.tensor_copy(h16[:], h0p_sb[:, :, None].to_broadcast([128, NK, B]))

            gi_A = singles.tile([128, NG, B, TBS], F16)
            gi_B = singles.tile([128, NG, B, TBS], F16)
            gi_hbm_r = gi_hbm.ap()  # [NTB, 128, NG, B*TBS]

            def load_gi(dst, blk):
                nc.sync.dma_start(
                    dst[:].rearrange("p j b t -> p j (b t)"), gi_hbm_r[blk]
                )

            h16f = h16[:].rearrange("p c b -> p (c b)")

            def emit_step(gi, tt, obt, oslot):
                """One GRU step; writes transposed new-h into obt[:, oslot, :]."""
                psr = ps_rz.tile([128, 8, B], F32, tag="psr")
                psn = ps_n.tile([128, 4, B], F32, tag="psn")
                psn2 = ps_n2.tile([128, 4, B], F32, tag="psn2")

                # bias injects (selector matmuls), then gi injects (identity)
                nc.tensor.matmul(psr[:], b_rz_sb[:], sel8_sb[:], start=True, stop=False)
                nc.tensor.matmul(psn[:], b_nh_sb[:], sel4_sb[:], start=True, stop=False)
                nc.tensor.matmul(psn2[:], b_ni_sb[:], sel4_sb[:], start=True, stop=False)
                nc.tensor.matmul(psr[:], ident_sb[:], gi[:, 0:8, :, tt], start=False, stop=False)
                nc.tensor.matmul(psn2[:], ident_sb[:], gi[:, 8:12, :, tt], start=False, stop=True)

                # recurrent matmuls: rz gates first, then n gate
                for j in range(8):
                    for k in range(NK):
                        nc.tensor.matmul(
                            psr[:, j],
                            whh_sb[:, j * NK + k],
                            h16[:, k],
                            start=False,
                            stop=(k == NK - 1),
                        )
                for j in range(4):
                    for k in range(NK):
                        nc.tensor.matmul(
                            psn[:, j],
                            whh_sb[:, (8 + j) * NK + k],
                            h16[:, k],
                            start=False,
                            stop=(k == NK - 1),
                        )

                # gate chain
                rz = chain.tile([128, 8, B], F32, tag="rz")
                nc.scalar.activation(rz[:], psr[:], AF.Sigmoid)
                u = chain.tile([128, 4, B], F32, tag="u")
                nc.vector.tensor_scalar(u[:], rz[:, 4:8], -1.0, 1.0, OP.mult, OP.add)
                e = chain.tile([128, 4, B], F32, tag="e")
                nc.vector.tensor_tensor(e[:], rz[:, 4:8], h32[:], OP.mult)
                t1 = chain.tile([128, 4, B], F32, tag="t1")
                nc.vector.tensor_tensor(t1[:], rz[:, 0:4], psn[:], OP.mult)
                t2 = chain.tile([128, 4, B], F32, tag="t2")
                nc.vector.tensor_tensor(t2[:], t1[:], psn2[:], OP.add)
                nt = chain.tile([128, 4, B], F32, tag="nt")
                nc.scalar.activation(nt[:], t2[:], AF.Tanh)
                t4 = chain.tile([128, 4, B], F32, tag="t4")
                nc.vector.tensor_tensor(t4[:], nt[:], u[:], OP.mult)
                hr = chain.tile([128, 4, B], F32, tag="hr")
                nc.vector.tensor_tensor(hr[:], t4[:], e[:], OP.add)
                nc.vector.tensor_scalar(h16[:], hr[:], CLIP, -CLIP, OP.min, OP.max)
                nc.vector.tensor_scalar(h32[:], hr[:], CLIP, -CLIP, OP.min, OP.max)

                # transposed output row: obt[:, oslot, :] = new h
                ptr = ps_tr.tile([64, 128], F32, tag="ptr")
                nc.tensor.transpose(ptr[:], h16f, ident_sb[:])
                nc.scalar.activation(obt[:, oslot], ptr[:], AF.Copy)

            def store_obt(obt, t0):
                # obt [64, TBS, 128] -> out rows t0..t0+TBS-1
                for c in range(NK):
                    nc.sync.dma_start(
                        out_cbtp[c, :, ds(t0, TBS), :] if isinstance(t0, int)
                        else out_cbtp[c, :, ds(t0, TBS), :],
                        obt[c * B : (c + 1) * B],
                    )

            # ---- prologue: t-block 0 (uses gi_A) ----
            load_gi(gi_A, 0)
            obt = obuf.tile([64, TBS, 128], F32, tag="obt")
            for tt in range(TBS):
                emit_step(gi_A, tt, obt, tt)
            store_obt(obt, 0)
            load_gi(gi_B, 1)

            # ---- main loop: 15 iterations x 2 t-blocks ----
            with tc.For_i(0, 15) as i:
                blk_even = 2 * i + 2
                load_gi(gi_A, blk_even)
                obtA = obuf.tile([64, TBS, 128], F32, tag="obt")
                for tt in range(TBS):
                    emit_step(gi_B, tt, obtA, tt)
                store_obt(obtA, i * 64 + 32)
                blk_odd = 2 * i + 3
                load_gi(gi_B, blk_odd)
                obtB = obuf.tile([64, TBS, 128], F32, tag="obt")
                for tt in range(TBS):
                    emit_step(gi_A, tt, obtB, tt)
                store_obt(obtB, i * 64 + 64)

            # ---- epilogue: t-block 31 (uses gi_B) ----
            obtZ = obuf.tile([64, TBS, 128], F32, tag="obt")
            for tt in range(TBS):
                emit_step(gi_B, tt, obtZ, tt)
            store_obt(obtZ, T - TBS)

    nc.compile()
    return nc


def _get_nc():
    if "nc" not in _cache:
        nc = bacc.Bacc("TRN2", target_bir_lowering=False, debug=False)
        _cache["nc"] = build(nc)
    return _cache["nc"]


def _prep_shared(h0, w_ih, w_hh, b_ih, b_hh):
    f16 = np.float16
    whhT = np.ascontiguousarray(w_hh.T).astype(f16)  # [H, 3H]
    whh_tiles = np.empty((128, 48, 128), f16)
    for j in range(NG):
        for k in range(NK):
            whh_tiles[:, j * NK + k, :] = whhT[
                k * 128 : (k + 1) * 128, j * 128 : (j + 1) * 128
            ]
    wihT = np.ascontiguousarray(w_ih.T).astype(f16)  # [I, 3H]
    wih_a = np.stack([wihT[:128], wihT[128:]], axis=0).transpose(1, 0, 2)
    wih_a = np.ascontiguousarray(wih_a)  # [128, 2, 3H]

    b_rz = (b_ih[: 2 * H] + b_hh[: 2 * H]).reshape(8, 128).astype(f16)
    b_nh = b_hh[2 * H :].reshape(4, 128).astype(f16)
    b_ni = b_ih[2 * H :].reshape(4, 128).astype(f16)
    sel8 = np.kron(np.eye(8), np.ones((1, B))).reshape(8, 8 * B).astype(f16)
    sel4 = np.kron(np.eye(4), np.ones((1, B))).reshape(4, 4 * B).astype(f16)
    ident = np.eye(128, dtype=f16)
    h0p = np.ascontiguousarray(h0.reshape(NK, 128).T).astype(np.float32)  # [p, c]
    return dict(
        h0p=h0p, whh=whh_tiles, wih=wih_a, b_rz=b_rz, b_nh=b_nh, b_ni=b_ni,
        sel8=sel8, sel4=sel4, ident=ident,
    )


def kernel(x, h0, w_ih, w_hh, b_ih, b_hh):
    x = np.asarray(x)
    shared = _prep_shared(
        np.asarray(h0), np.asarray(w_ih), np.asarray(w_hh),
        np.asarray(b_ih), np.asarray(b_hh),
    )
    nc = _get_nc()
    in_maps = []
    for c in range(N_CORES):
        m = dict(shared)
        m["x16"] = np.ascontiguousarray(x[c * B : (c + 1) * B]).astype(np.float16)
        in_maps.append(m)
    res = run_bass_kernel_spmd(nc, in_maps, core_ids=list(range(N_CORES)))
    output = np.concatenate([res.results[c]["out"] for c in range(N_CORES)], axis=0)
    output = output.astype(np.float32)
    h_last = np.ascontiguousarray(output[:, -1, :])
    return output, h_last


# revision 11
# speedup vs baseline: 8372.8436x; 8372.8436x over previous
"""Trainium2 Bass kernel for nn_ClippedGRU: B=128, T=1024, I=256, H=512, clip=5.0.

Strategy (8-way data-parallel, B=16 per core, no collectives):
  All on-chip state lives in "transposed" layout: gates/hidden on SBUF
  partitions, (batch, time) on the free dim.

  Phase 1: gi.T = w_ih @ x.T as one big PE GEMM (fp16 in, fp32 PSUM),
  staged to an HBM scratch in per-t-block tiles [128, 12, (16b x 32t)].
  x.T comes from xbar DMA-transpose (fp16).

  Phase 2: 1024 serial GRU steps. Per step:
    - 3 tiny "bias inject" matmuls put b_hh/b_ih into PSUM (selector trick)
    - 1-2 "identity inject" matmuls accumulate gi_t into PSUM
    - 48 LDWEIGHTS+MATMUL pairs (w_hh.T tiles fp16, N=16) -> gates PSUM
    - ACT sigmoid/tanh + DVE elementwise chain -> h (fp32 master + fp16 copy)
    - PE transpose of h -> output rows staged [64, 32t, 128] and DMA'd per block
"""

import sys
import os

for _p in ("/opt/trn_rl_repo", "/root/.axon_site/_ro/trn_rl_repo"):
    if os.path.isdir(_p) and _p not in sys.path:
        sys.path.insert(0, _p)

import numpy as np

import concourse.bass as bass
import concourse.tile as tile
from concourse import bacc, mybir
from concourse.bass import ds, ts
from concourse.bass_utils import run_bass_kernel_spmd

F16 = mybir.dt.float16
F32 = mybir.dt.float32
AF = mybir.ActivationFunctionType
OP = mybir.AluOpType

B_FULL, T, I, H = 128, 1024, 256, 512
N_CORES = 8
B = B_FULL // N_CORES        # 16 per core
TBS = 32                     # timesteps per t-block
NG = 12                      # gate chunks of 128 (3H/128)
NK = H // 128                # 4 k-chunks of hidden
CLIP = 5.0

_cache = {}


def _emit_phase1(nc, tc, ctx, x16, wih, gi_hbm, T_loc):
    """gi.T = w_ih @ x.T  ->  gi_hbm[tb, p, j, (b t)]  (fp16)."""
    import contextlib

    with contextlib.ExitStack() as p1:
        pool = p1.enter_context(tc.tile_pool(name="p1sbuf", bufs=1))
        tmp = p1.enter_context(tc.tile_pool(name="p1tmp", bufs=4))
        psum = p1.enter_context(tc.tile_pool(name="p1psum", bufs=4, space="PSUM"))

        NTB = T_loc // TBS
        wih_sb = pool.tile([128, 2, 3 * H], F16)
        nc.sync.dma_start(wih_sb[:], wih[:])

        xT = pool.tile([128, 2, B * T_loc], F16)
        for b in range(B):
            for k in range(2):
                nc.sync.dma_start_transpose(
                    xT[:, k, b * T_loc : (b + 1) * T_loc],
                    x16[b, :, k * 128 : (k + 1) * 128],
                )

        xTr = [xT[:, k].rearrange("p (b t) -> p b t", b=B) for k in range(2)]
        for tb in range(NTB):
            for j in range(NG):
                ps = psum.tile([128, B * TBS], F32, tag="p1ps")
                for k in range(2):
                    nc.tensor.matmul(
                        ps[:],
                        wih_sb[:, k, ts(j, 128)],
                        xTr[k][:, :, ts(tb, TBS)],
                        start=(k == 0),
                        stop=(k == 1),
                    )
                gi_t = tmp.tile([128, B * TBS], F16, tag="p1gi")
                if j % 2 == 0:
                    nc.scalar.activation(gi_t[:], ps[:], AF.Copy)
                else:
                    nc.vector.tensor_copy(gi_t[:], ps[:])
                nc.sync.dma_start(gi_hbm[tb, :, j, :], gi_t[:])


def build(nc, T_loc=T, unroll_all=False):
    NTB = T_loc // TBS
    x16 = nc.declare_dram_parameter("x16", [B, T_loc, I], F16, isOutput=False)
    h0p = nc.declare_dram_parameter("h0p", [128, NK], F32, isOutput=False)
    whh = nc.declare_dram_parameter("whh", [128, 48, 128], F16, isOutput=False)
    wih = nc.declare_dram_parameter("wih", [128, 2, 3 * H], F16, isOutput=False)
    b_rz = nc.declare_dram_parameter("b_rz", [8, 128], F16, isOutput=False)
    b_nh = nc.declare_dram_parameter("b_nh", [4, 128], F16, isOutput=False)
    b_ni = nc.declare_dram_parameter("b_ni", [4, 128], F16, isOutput=False)
    sel8 = nc.declare_dram_parameter("sel8", [8, 128], F16, isOutput=False)
    sel4 = nc.declare_dram_parameter("sel4", [4, 64], F16, isOutput=False)
    ident = nc.declare_dram_parameter("ident", [128, 128], F16, isOutput=False)
    out = nc.declare_dram_parameter("out", [B, T_loc, H], F32, isOutput=True)

    gi_hbm = nc.dram_tensor("gi_scratch", [NTB, 128, NG, B * TBS], F16)

    # output viewed as [c, b, t, p] where h = c*128 + p
    out_cbtp = out.ap().rearrange("b t (c p) -> c b t p", p=128)

    with tile.TileContext(nc) as tc:
        import contextlib

        ctx = contextlib.ExitStack()
        with ctx:
            _emit_phase1(nc, tc, ctx, x16, wih, gi_hbm, T_loc)

            singles = ctx.enter_context(tc.tile_pool(name="singles", bufs=1))
            chain = ctx.enter_context(tc.tile_pool(name="chain", bufs=2))
            obuf = ctx.enter_context(tc.tile_pool(name="obuf", bufs=2))
            ps_rz = ctx.enter_context(tc.tile_pool(name="ps_rz", bufs=2, space="PSUM"))
            ps_n = ctx.enter_context(tc.tile_pool(name="ps_n", bufs=2, space="PSUM"))
            ps_n2 = ctx.enter_context(tc.tile_pool(name="ps_n2", bufs=2, space="PSUM"))
            ps_tr = ctx.enter_context(tc.tile_pool(name="ps_tr", bufs=2, space="PSUM"))

            whh_sb = singles.tile([128, 48, 128], F16)
            nc.sync.dma_start(whh_sb[:], whh[:])
            ident_sb = singles.tile([128, 128], F16)
            nc.sync.dma_start(ident_sb[:], ident[:])
            sel8_sb = singles.tile([8, 128], F16)
            nc.sync.dma_start(sel8_sb[:], sel8[:])
            sel4_sb = singles.tile([4, 64], F16)
            nc.sync.dma_start(sel4_sb[:], sel4[:])
            b_rz_sb = singles.tile([8, 128], F16)
            nc.sync.dma_start(b_rz_sb[:], b_rz[:])
            b_nh_sb = singles.tile([4, 128], F16)
            nc.sync.dma_start(b_nh_sb[:], b_nh[:])
            b_ni_sb = singles.tile([4, 128], F16)
            nc.sync.dma_start(b_ni_sb[:], b_ni[:])

            h16 = singles.tile([128, NK, B], F16)
            h32 = singles.tile([128, NK, B], F32)
            h0p_sb = singles.tile([128, NK], F32)
            nc.sync.dma_start(h0p_sb[:], h0p[:])
            nc.vector.tensor_copy(h32[:], h0p_sb[:, :, None].to_broadcast([128, NK, B]))
            nc.vector.tensor_copy(h16[:], h0p_sb[:, :, None].to_broadcast([128, NK, B]))

            gi_A = singles.tile([128, NG, B, TBS], F16)
            gi_B = singles.tile([128, NG, B, TBS], F16)
            gi_hbm_r = gi_hbm.ap()  # [NTB, 128, NG, B*TBS]

            def load_gi(dst, blk):
                nc.sync.dma_start(
                    dst[:].rearrange("p j b t -> p j (b t)"), gi_hbm_r[blk]
                )

            h16f = h16[:].rearrange("p c b -> p (c b)")

            def emit_step(gi, tt, obt, oslot):
                """One GRU step; writes transposed new-h into obt[:, oslot, :]."""
                psr = ps_rz.tile([128, 8, B], F32, tag="psr")
                psn = ps_n.tile([128, 4, B], F32, tag="psn")
                psn2 = ps_n2.tile([128, 4, B], F32, tag="psn2")

                # bias injects (selector matmuls), then gi injects (identity)
                nc.tensor.matmul(psr[:], b_rz_sb[:], sel8_sb[:], start=True, stop=False)
                nc.tensor.matmul(psn[:], b_nh_sb[:], sel4_sb[:], start=True, stop=False)
                nc.tensor.matmul(psn2[:], b_ni_sb[:], sel4_sb[:], start=True, stop=False)
                nc.tensor.matmul(psr[:], ident_sb[:], gi[:, 0:8, :, tt], start=False, stop=False)
                nc.tensor.matmul(psn2[:], ident_sb[:], gi[:, 8:12, :, tt], start=False, stop=True)

                # recurrent matmuls: rz gates first, then n gate
                for j in range(8):
                    for k in range(NK):
                        nc.tensor.matmul(
                            psr[:, j],
                            whh_sb[:, j * NK + k],
                            h16[:, k],
                            start=False,
                            stop=(k == NK - 1),
                        )
                for j in range(4):
                    for k in range(NK):
                        nc.tensor.matmul(
                            psn[:, j],
                            whh_sb[:, (8 + j) * NK + k],
                            h16[:, k],
                            start=False,
                            stop=(k == NK - 1),
                        )

                # gate chain
                rz = chain.tile([128, 8, B], F32, tag="rz")
                nc.scalar.activation(rz[:], psr[:], AF.Sigmoid)
                u = chain.tile([128, 4, B], F32, tag="u")
                nc.vector.tensor_scalar(u[:], rz[:, 4:8], -1.0, 1.0, OP.mult, OP.add)
                e = chain.tile([128, 4, B], F32, tag="e")
                nc.vector.tensor_tensor(e[:], rz[:, 4:8], h32[:], OP.mult)
                t1 = chain.tile([128, 4, B], F32, tag="t1")
                nc.vector.tensor_tensor(t1[:], rz[:, 0:4], psn[:], OP.mult)
                t2 = chain.tile([128, 4, B], F32, tag="t2")
                nc.vector.tensor_tensor(t2[:], t1[:], psn2[:], OP.add)
                nt = chain.tile([128, 4, B], F32, tag="nt")
                nc.scalar.activation(nt[:], t2[:], AF.Tanh)
                t4 = chain.tile([128, 4, B], F32, tag="t4")
                nc.vector.tensor_tensor(t4[:], nt[:], u[:], OP.mult)
                hr = chain.tile([128, 4, B], F32, tag="hr")
                nc.vector.tensor_tensor(hr[:], t4[:], e[:], OP.add)
                nc.vector.tensor_scalar(h16[:], hr[:], CLIP, -CLIP, OP.min, OP.max)
                nc.vector.tensor_scalar(h32[:], hr[:], CLIP, -CLIP, OP.min, OP.max)

                # transposed output row: obt[:, oslot, :] = new h
                ptr = ps_tr.tile([64, 128], F16, tag="ptr")
                nc.tensor.transpose(ptr[:], h16f, ident_sb[:])
                nc.scalar.activation(obt[:, oslot], ptr[:], AF.Copy)

            def store_obt(obt, t0):
                # obt [64, TBS, 128] -> out rows t0..t0+TBS-1
                for c in range(NK):
                    nc.sync.dma_start(
                        out_cbtp[c, :, ds(t0, TBS), :],
                        obt[c * B : (c + 1) * B],
                    )

            if unroll_all:
                # fully static (for cost-model timeline runs at small T)
                gis = [gi_A, gi_B]
                load_gi(gi_A, 0)
                for tb in range(NTB):
                    if tb + 1 < NTB:
                        load_gi(gis[(tb + 1) % 2], tb + 1)
                    obt = obuf.tile([64, TBS, 128], F32, tag="obt")
                    for tt in range(TBS):
                        emit_step(gis[tb % 2], tt, obt, tt)
                    store_obt(obt, tb * TBS)
            else:
                # ---- prologue: t-block 0 (uses gi_A) ----
                load_gi(gi_A, 0)
                obt = obuf.tile([64, TBS, 128], F32, tag="obt")
                for tt in range(TBS):
                    emit_step(gi_A, tt, obt, tt)
                store_obt(obt, 0)
                load_gi(gi_B, 1)

                # ---- main loop: (NTB-2)/2 iterations x 2 t-blocks ----
                with tc.For_i(0, (NTB - 2) // 2) as i:
                    blk_even = 2 * i + 2
                    load_gi(gi_A, blk_even)
                    obtA = obuf.tile([64, TBS, 128], F32, tag="obt")
                    for tt in range(TBS):
                        emit_step(gi_B, tt, obtA, tt)
                    store_obt(obtA, i * 64 + 32)
                    blk_odd = 2 * i + 3
                    load_gi(gi_B, blk_odd)
                    obtB = obuf.tile([64, TBS, 128], F32, tag="obt")
                    for tt in range(TBS):
                        emit_step(gi_A, tt, obtB, tt)
                    store_obt(obtB, i * 64 + 64)

                # ---- epilogue: last t-block (uses gi_B) ----
                obtZ = obuf.tile([64, TBS, 128], F32, tag="obt")
                for tt in range(TBS):
                    emit_step(gi_B, tt, obtZ, tt)
                store_obt(obtZ, T_loc - TBS)

    nc.compile()
    return nc


def _get_nc():
    if "nc" not in _cache:
        T_loc = int(os.environ.get("KERNEL_T", str(T)))
        unroll_all = bool(int(os.environ.get("KERNEL_UNROLL_ALL", "0")))
        nc = bacc.Bacc("TRN2", target_bir_lowering=False, debug=False)
        _cache["nc"] = build(nc, T_loc=T_loc, unroll_all=unroll_all)
    return _cache["nc"]


def _prep_shared(h0, w_ih, w_hh, b_ih, b_hh):
    f16 = np.float16
    whhT = np.ascontiguousarray(w_hh.T).astype(f16)  # [H, 3H]
    whh_tiles = np.empty((128, 48, 128), f16)
    for j in range(NG):
        for k in range(NK):
            whh_tiles[:, j * NK + k, :] = whhT[
                k * 128 : (k + 1) * 128, j * 128 : (j + 1) * 128
            ]
    wihT = np.ascontiguousarray(w_ih.T).astype(f16)  # [I, 3H]
    wih_a = np.stack([wihT[:128], wihT[128:]], axis=0).transpose(1, 0, 2)
    wih_a = np.ascontiguousarray(wih_a)  # [128, 2, 3H]

    b_rz = (b_ih[: 2 * H] + b_hh[: 2 * H]).reshape(8, 128).astype(f16)
    b_nh = b_hh[2 * H :].reshape(4, 128).astype(f16)
    b_ni = b_ih[2 * H :].reshape(4, 128).astype(f16)
    sel8 = np.kron(np.eye(8), np.ones((1, B))).reshape(8, 8 * B).astype(f16)
    sel4 = np.kron(np.eye(4), np.ones((1, B))).reshape(4, 4 * B).astype(f16)
    ident = np.eye(128, dtype=f16)
    h0p = np.ascontiguousarray(h0.reshape(NK, 128).T).astype(np.float32)  # [p, c]
    return dict(
        h0p=h0p, whh=whh_tiles, wih=wih_a, b_rz=b_rz, b_nh=b_nh, b_ni=b_ni,
        sel8=sel8, sel4=sel4, ident=ident,
    )


def kernel(x, h0, w_ih, w_hh, b_ih, b_hh):
    x = np.asarray(x)
    shared = _prep_shared(
        np.asarray(h0), np.asarray(w_ih), np.asarray(w_hh),
        np.asarray(b_ih), np.asarray(b_hh),
    )
    nc = _get_nc()
    in_maps = []
    for c in range(N_CORES):
        m = dict(shared)
        m["x16"] = np.ascontiguousarray(x[c * B : (c + 1) * B]).astype(np.float16)
        in_maps.append(m)
    res = run_bass_kernel_spmd(nc, in_maps, core_ids=list(range(N_CORES)))
    _cache["last_res"] = res
    output = np.concatenate([res.results[c]["out"] for c in range(N_CORES)], axis=0)
    output = output.astype(np.float32)
    h_last = np.ascontiguousarray(output[:, -1, :])
    return output, h_last
